# revision 27
# baseline (speedup 1.0000x reference)
"""Trainium2 Bass kernel for nn_AFMADEBlock (autoregressive flow MADE block).

Reference semantics (D=32 sequential fixed-point steps over batch B=4096):
    for step in range(32):
        mu = made_mu(y); lv = made_lv(y)           # two masked MLPs 32->256->256->32
        y  = (x - mu) / (exp(0.5*lv) + 1e-12)
    return y, (0.5*lv).sum(axis=1)                 # lv from the LAST step

Sharding: pure data parallel -- batch 4096 -> 8 cores x 512, weights replicated.

Kernel strategy per core:
- Feature-on-partition layout: yT [32, 512]; hidden layers [256, 512] as two
  128-partition chunks.  All matmuls have moving dim N=512 (f32r dtype: full
  rate fp32 on the PE, ~1.5e-4 matmul rel err).
- Hidden units permuted by MADE degree so chunk0 = degrees 1..15 (128 units),
  chunk1 = degrees 16..31.  The W1*M1 block (chunk1 -> chunk0) is then exactly
  zero (3 matmuls instead of 4 in layer 2), and steps 0..14 only need chunk0
  anywhere (y coords >= step stay garbage, exactly like the reference
  carries them).
- Biases: b0/b1 folded into the relu ops (ACT: activation bias; DVE:
  tensor_scalar add+max); b2_mu folded into x1 = xT - b2mu; b2_lv folded into
  the exp bias.
- mu-branch output weights negated and x1 accumulated into the same PSUM via
  an identity matmul, so PSUM directly holds (x - mu).
- y = (x - mu) * exp(-0.5*lv - 0.5*b2lv): eps=1e-12 is negligible (|lv| < 1).
- PE warmup burst + Exp-table preload during the weight-DMA phase.
"""

import numpy as np

D = 32
H = 256
B = 4096
NCORES = 8
BS = B // NCORES          # 512 batch per core
C0 = 128                  # hidden chunk size
NB = BS // C0             # 4 batch chunks of 128 (for transposes)


# ---------------------------------------------------------------- host prep --
def _masks():
    deg_in = np.arange(1, D + 1)
    deg_h = (np.arange(H) % (D - 1)) + 1
    m0 = (deg_h[None, :] >= deg_in[:, None]).astype(np.float32)
    m1 = (deg_h[None, :] >= deg_h[:, None]).astype(np.float32)
    m2 = (deg_in[None, :] > deg_h[:, None]).astype(np.float32)
    perm = np.argsort(deg_h, kind="stable")
    return m0, m1, m2, perm


def _prep_weights(inputs):
    """Mask, permute (degree-sorted), and lay out weights for the kernel."""
    m0, m1, m2, perm = _masks()
    out = {}
    for br in ("mu", "lv"):
        W0 = np.asarray(inputs[f"{br}_W0"], np.float32)
        W1 = np.asarray(inputs[f"{br}_W1"], np.float32)
        W2 = np.asarray(inputs[f"{br}_W2"], np.float32)
        b0 = np.asarray(inputs[f"{br}_b0"], np.float32)
        b1 = np.asarray(inputs[f"{br}_b1"], np.float32)
        b2 = np.asarray(inputs[f"{br}_b2"], np.float32)
        W0m = (W0 * m0)[:, perm]                     # [32, 256]
        W1m = (W1 * m1)[perm][:, perm]               # [256, 256]
        W2m = (W2 * m2)[perm, :]                     # [256, 32]
        out[f"{br}_w0"] = np.ascontiguousarray(W0m)
        out[f"{br}_w1a"] = np.ascontiguousarray(W1m[:C0, :])        # [128, 256]
        out[f"{br}_w1b"] = np.ascontiguousarray(W1m[C0:, C0:])      # [128, 128]
        sgn = -1.0 if br == "mu" else 1.0
        out[f"{br}_w2a"] = np.ascontiguousarray(sgn * W2m[:C0, :])  # [128, 32]
        out[f"{br}_w2b"] = np.ascontiguousarray(sgn * W2m[C0:, :])  # [128, 32]
        out[f"{br}_b0"] = np.ascontiguousarray(b0[perm]).reshape(H, 1)
        out[f"{br}_b1"] = np.ascontiguousarray(b1[perm]).reshape(H, 1)
        out[f"{br}_b2"] = np.ascontiguousarray(b2).reshape(D, 1)
    # 0.5 * sum(b2_lv): host-side constant added to the logstd reduction
    out["lsb2"] = np.asarray(
        [[0.5 * float(np.sum(np.asarray(inputs["lv_b2"], np.float32)))]],
        np.float32,
    )
    return out


_WEIGHT_SPECS = [
    ("mu_w0", [D, H]), ("mu_w1a", [C0, H]), ("mu_w1b", [C0, C0]),
    ("mu_w2a", [C0, D]), ("mu_w2b", [C0, D]),
    ("mu_b0", [H, 1]), ("mu_b1", [H, 1]), ("mu_b2", [D, 1]),
    ("lv_w0", [D, H]), ("lv_w1a", [C0, H]), ("lv_w1b", [C0, C0]),
    ("lv_w2a", [C0, D]), ("lv_w2b", [C0, D]),
    ("lv_b0", [H, 1]), ("lv_b1", [H, 1]), ("lv_b2", [D, 1]),
    ("lsb2", [1, 1]),
]


# ------------------------------------------------------------- bass builder --
def build_bass(nsteps=D):
    import concourse.bacc as bacc
    import concourse.mybir as mybir
    from concourse.tile import TileContext
    from concourse.masks import make_identity

    f32 = mybir.dt.float32
    f32r = mybir.dt.float32r
    Relu = mybir.ActivationFunctionType.Relu
    Exp = mybir.ActivationFunctionType.Exp
    Copy = mybir.ActivationFunctionType.Copy
    add_op = mybir.AluOpType.add
    max_op = mybir.AluOpType.max
    mult_op = mybir.AluOpType.mult
    sub_op = mybir.AluOpType.subtract

    nc = bacc.Bacc("TRN2", target_bir_lowering=False, debug=False,
                   num_devices=NCORES)

    x_d = nc.declare_dram_parameter("x", [BS, D], f32, isOutput=False)

    def _wdt(nm):
        return f32r if nm.split("_")[-1] in ("w0", "w1a", "w1b", "w2a",
                                             "w2b") else f32
    wd = {nm: nc.declare_dram_parameter(nm, shp, _wdt(nm), isOutput=False)
          for nm, shp in _WEIGHT_SPECS}
    identr_d = nc.declare_dram_parameter("identr", [D, D], f32r, isOutput=False)
    y_d = nc.declare_dram_parameter("y_out", [BS, D], f32, isOutput=True)
    ls_d = nc.declare_dram_parameter("ls_out", [1, BS], f32, isOutput=True)

    with (
        TileContext(nc) as tc,
        tc.tile_pool(name="consts", bufs=1) as cpool,
        tc.tile_pool(name="acts", bufs=1) as apool,
        tc.tile_pool(name="hsb", bufs=4) as hpool,
        tc.tile_pool(name="pmm", bufs=5, space="PSUM") as pmm,
        tc.tile_pool(name="pout", bufs=1, space="PSUM") as pout,
        tc.tile_pool(name="pmisc", bufs=1, space="PSUM") as pmisc,
    ):
        # ---- constants ----
        ident = cpool.tile([C0, C0], f32, tag="ident")
        make_identity(nc, ident[:])
        w = {}
        for nm, shp in _WEIGHT_SPECS:
            if nm.endswith("b0") or nm.endswith("b1"):
                # [256,1] bias -> [128,2]: col 0 = chunk0, col 1 = chunk1
                t = cpool.tile([C0, 2], f32, tag=nm)
                nc.sync.dma_start(
                    out=t[:], in_=wd[nm].rearrange("(c p) o -> p (c o)", c=2))
            else:
                t = cpool.tile(shp, _wdt(nm), tag=nm)
                nc.sync.dma_start(out=t[:], in_=wd[nm][:])
            w[nm] = t
        identr = cpool.tile([D, D], f32r, tag="identr")
        nc.sync.dma_start(out=identr[:], in_=identr_d[:])

        # exp bias: -0.5 * b2_lv  (per-partition [32,1])
        expb = cpool.tile([D, 1], f32, tag="expb")
        nc.scalar.mul(out=expb[:], in_=w["lv_b2"][:], mul=-0.5)

        # 0.5-vector for the final logstd partition reduction
        halves = cpool.tile([D, 1], f32r, tag="halves")
        nc.vector.memset(halves[:].bitcast(f32), 0.5)

        # ---- x: load natural, PE-transpose to xT, fold b2_mu: x1 = xT - b2mu
        x_nat = apool.tile([C0, NB * D], f32, tag="xnat")
        nc.sync.dma_start(
            out=x_nat[:].rearrange("p (c d) -> p c d", c=NB),
            in_=x_d.rearrange("(c p) d -> p c d", p=C0),
        )
        x1 = apool.tile([D, BS], f32r, tag="x1")
        for i in range(NB):
            tp = pmisc.tile([D, C0], f32, tag="misc")
            nc.tensor.transpose(tp[:], x_nat[:, i * D:(i + 1) * D], ident[:])
            nc.vector.tensor_scalar(
                out=x1[:, i * C0:(i + 1) * C0], in0=tp[:],
                scalar1=w["mu_b2"][:], scalar2=None, op0=sub_op,
            )

        # ---- yT state [32, 512], init 0
        yt = apool.tile([D, BS], f32r, tag="yt")
        nc.vector.memset(yt[:].bitcast(f32), 0.0)

        # ---- PE warmup burst (opens the HAM clock gate before step 0) and
        #      Exp activation-table preload, both during the DMA phase.
        zw = cpool.tile([D, C0], f32r, tag="zw")
        nc.vector.memset(zw[:].bitcast(f32), 0.0)
        wrm = pmm.tile([C0, BS], f32, tag="hps")
        for i in range(20):
            r = i % 2
            nc.tensor.matmul(wrm[:, r * 256:(r + 1) * 256], zw[:], x1[:, :256],
                             start=True, stop=True)
        pre_e = hpool.tile([D, 1], f32, tag="pre_e")
        nc.scalar.activation(pre_e[:], expb[:], Exp)

        lv_ps_last = None
        HB = BS // 2

        # ---------------- the 32 steps ----------------
        # Steps 0..14 only need hidden units with degree <= step (chunk0):
        # output coord k depends on h2 units deg<=k -> h1 units deg<=k ->
        # y coords < k.  Coordinates > k of the y update are garbage-but-
        # carried, exactly like the reference.
        for step in range(nsteps):
            full = step >= 15
            chunks = (0, 1) if full else (0,)

            # mm1: h1T chunks = W0.T @ yT   (K=32, M=128, N=512)
            h1_ps = {}
            for c in chunks:
                for br in ("mu", "lv"):
                    ps = pmm.tile([C0, BS], f32, tag="hps")
                    nc.tensor.matmul(
                        ps[:],
                        w[f"{br}_w0"][:, c * C0:(c + 1) * C0],
                        yt[:],
                        start=True, stop=True,
                    )
                    h1_ps[br, c] = ps

            # relu1 (+b0): mu_c0/lv_c1 on ACT, lv_c0/mu_c1 on DVE
            h1 = {}
            for c in chunks:
                for br in ("mu", "lv"):
                    t = hpool.tile([C0, BS], f32r, tag="h1")
                    on_act = (br == "mu") == (c == 0)
                    if on_act:
                        nc.scalar.activation(
                            t[:], h1_ps[br, c][:], Relu,
                            bias=w[f"{br}_b0"][:, c:c + 1], scale=1.0,
                        )
                    else:
                        nc.vector.tensor_scalar(
                            out=t[:], in0=h1_ps[br, c][:],
                            scalar1=w[f"{br}_b0"][:, c:c + 1], scalar2=0.0,
                            op0=add_op, op1=max_op,
                        )
                    h1[br, c] = t

            # mm2: a2c0 = w1a[:, :128].T @ h1c0
            #      a2c1 = w1a[:, 128:].T @ h1c0 + w1b.T @ h1c1  (full only)
            h2_ps = {}
            for br in ("lv", "mu"):
                ps0 = pmm.tile([C0, BS], f32, tag="hps")
                nc.tensor.matmul(
                    ps0[:], w[f"{br}_w1a"][:, :C0],
                    h1[br, 0][:], start=True, stop=True,
                )
                h2_ps[br, 0] = ps0
            if full:
                for br in ("lv", "mu"):
                    ps1 = pmm.tile([C0, BS], f32, tag="hps")
                    nc.tensor.matmul(
                        ps1[:], w[f"{br}_w1a"][:, C0:],
                        h1[br, 0][:], start=True, stop=False,
                    )
                    nc.tensor.matmul(
                        ps1[:], w[f"{br}_w1b"][:],
                        h1[br, 1][:], start=False, stop=True,
                    )
                    h2_ps[br, 1] = ps1

            # relu2 (+b1)
            h2 = {}
            for c in chunks:
                for br in ("lv", "mu"):
                    t = hpool.tile([C0, BS], f32r, tag="h2")
                    on_act = (br == "mu") == (c == 0)
                    if on_act:
                        nc.scalar.activation(
                            t[:], h2_ps[br, c][:], Relu,
                            bias=w[f"{br}_b1"][:, c:c + 1], scale=1.0,
                        )
                    else:
                        nc.vector.tensor_scalar(
                            out=t[:], in0=h2_ps[br, c][:],
                            scalar1=w[f"{br}_b1"][:, c:c + 1], scalar2=0.0,
                            op0=add_op, op1=max_op,
                        )
                    h2[br, c] = t

            # mm3 lv first (exp is on the critical chain)
            lv_ps = pout.tile([D, BS], f32, tag="olv")
            nc.tensor.matmul(lv_ps[:], w["lv_w2a"][:],
                             h2["lv", 0][:], start=True, stop=not full)
            if full:
                nc.tensor.matmul(lv_ps[:], w["lv_w2b"][:],
                                 h2["lv", 1][:], start=False, stop=True)

            # mm3 mu: psum = x1 - mu  (negated W2mu, then +x1 via identity mm)
            tmu_ps = pout.tile([D, BS], f32, tag="omu")
            nc.tensor.matmul(tmu_ps[:], w["mu_w2a"][:],
                             h2["mu", 0][:], start=True, stop=False)
            if full:
                nc.tensor.matmul(tmu_ps[:], w["mu_w2b"][:],
                                 h2["mu", 1][:], start=False, stop=False)
            nc.tensor.matmul(tmu_ps[:], identr[:],
                             x1[:], start=False, stop=True)

            # tail in batch halves: e = exp(-0.5*lv - 0.5*b2lv); y = t * e
            e = hpool.tile([D, BS], f32, tag="e")
            for hh in range(2):
                sl = slice(hh * HB, (hh + 1) * HB)
                nc.scalar.activation(e[:, sl], lv_ps[:, sl], Exp,
                                     bias=expb[:], scale=-0.5)
                nc.vector.tensor_tensor(out=yt[:, sl], in0=tmu_ps[:, sl],
                                        in1=e[:, sl], op=mult_op)

            lv_ps_last = lv_ps

        # ---------------- outputs ----------------
        # logstd_sum = 0.5 * sum_d lv_last[d, b] + 0.5*sum(b2lv)
        lvt = hpool.tile([D, BS], f32r, tag="lvt")
        nc.scalar.activation(lvt[:], lv_ps_last[:], Copy)
        ls_ps = pmisc.tile([1, BS], f32, tag="misc")
        nc.tensor.matmul(ls_ps[:], halves[:], lvt[:], start=True, stop=True)
        ls_sb = hpool.tile([1, BS], f32, tag="lssb")
        nc.vector.tensor_scalar(
            out=ls_sb[:], in0=ls_ps[:], scalar1=w["lsb2"][0:1, 0:1],
            scalar2=None, op0=add_op,
        )
        nc.sync.dma_start(out=ls_d[:], in_=ls_sb[:])

        # y: PE-transpose yT [32,512] -> [512,32] and DMA out
        y_nat = hpool.tile([C0, NB * D], f32, tag="ynat")
        for i in range(NB):
            yps = pmisc.tile([C0, D], f32, tag="misc")
            nc.tensor.transpose(yps[:], yt[:, i * C0:(i + 1) * C0].bitcast(f32),
                                ident[:D, :D])
            nc.scalar.activation(y_nat[:, i * D:(i + 1) * D], yps[:], Copy)
        nc.sync.dma_start(
            out=y_d.rearrange("(c p) d -> p c d", p=C0),
            in_=y_nat[:].rearrange("p (c d) -> p c d", c=NB),
        )

    nc.finalize()
    return nc


# ------------------------------------------------------------ host entry ----
_CACHED = {}


def make_in_maps(inputs):
    x = np.asarray(inputs["x"], np.float32)
    wmaps = _prep_weights(inputs)
    ident = np.eye(D, dtype=np.float32)
    in_maps = []
    for c in range(NCORES):
        m = {"x": np.ascontiguousarray(x[c * BS:(c + 1) * BS]),
             "identr": ident}
        m.update(wmaps)
        in_maps.append(m)
    return in_maps


def kernel(**inputs):
    import concourse.bass_utils as bass_utils

    if "nc" not in _CACHED:
        _CACHED["nc"] = build_bass()
    nc = _CACHED["nc"]
    in_maps = make_in_maps(inputs)

    res = bass_utils.run_bass_kernel_spmd(nc, in_maps,
                                          core_ids=list(range(NCORES)))
    y = np.concatenate([res.results[c]["y_out"] for c in range(NCORES)], axis=0)
    ls = np.concatenate(
        [res.results[c]["ls_out"].reshape(BS) for c in range(NCORES)], axis=0
    )
    return y, ls


if __name__ == "__main__":
    nc = build_bass()
    print("built ok")


# revision 29
# speedup vs baseline: 1.2313x; 1.2313x over previous
"""Trainium2 Bass kernel for nn_AFMADEBlock (autoregressive flow MADE block).

Reference semantics (D=32 sequential fixed-point steps over batch B=4096):
    for step in range(32):
        mu = made_mu(y); lv = made_lv(y)           # two masked MLPs 32->256->256->32
        y  = (x - mu) / (exp(0.5*lv) + 1e-12)
    return y, (0.5*lv).sum(axis=1)                 # lv from the LAST step

Sharding: pure data parallel -- batch 4096 -> 8 cores x 512, weights replicated.

Kernel strategy per core:
- Feature-on-partition layout: yT [32, 512]; hidden layers [256, 512] as two
  128-partition chunks.  All matmuls have moving dim N=512 (f32r dtype: full
  rate fp32 on the PE, ~1.5e-4 matmul rel err).
- Hidden units permuted by MADE degree so chunk0 = degrees 1..15 (128 units),
  chunk1 = degrees 16..31.  The W1*M1 block (chunk1 -> chunk0) is then exactly
  zero (3 matmuls instead of 4 in layer 2), and steps 0..14 only need chunk0
  anywhere (y coords >= step stay garbage, exactly like the reference
  carries them).
- Biases: b0/b1 folded into the relu ops (ACT: activation bias; DVE:
  tensor_scalar add+max); b2_mu folded into x1 = xT - b2mu; b2_lv folded into
  the exp bias.
- mu-branch output weights negated and x1 accumulated into the same PSUM via
  an identity matmul, so PSUM directly holds (x - mu).
- y = (x - mu) * exp(-0.5*lv - 0.5*b2lv): eps=1e-12 is negligible (|lv| < 1).
- PE warmup burst + Exp-table preload during the weight-DMA phase.
"""

import numpy as np

D = 32
H = 256
B = 4096
NCORES = 8
BS = B // NCORES          # 512 batch per core
C0 = 128                  # hidden chunk size
NB = BS // C0             # 4 batch chunks of 128 (for transposes)


# ---------------------------------------------------------------- host prep --
def _masks():
    deg_in = np.arange(1, D + 1)
    deg_h = (np.arange(H) % (D - 1)) + 1
    m0 = (deg_h[None, :] >= deg_in[:, None]).astype(np.float32)
    m1 = (deg_h[None, :] >= deg_h[:, None]).astype(np.float32)
    m2 = (deg_in[None, :] > deg_h[:, None]).astype(np.float32)
    perm = np.argsort(deg_h, kind="stable")
    return m0, m1, m2, perm


def _prep_weights(inputs):
    """Mask, permute (degree-sorted), and lay out weights for the kernel."""
    m0, m1, m2, perm = _masks()
    out = {}
    for br in ("mu", "lv"):
        W0 = np.asarray(inputs[f"{br}_W0"], np.float32)
        W1 = np.asarray(inputs[f"{br}_W1"], np.float32)
        W2 = np.asarray(inputs[f"{br}_W2"], np.float32)
        b0 = np.asarray(inputs[f"{br}_b0"], np.float32)
        b1 = np.asarray(inputs[f"{br}_b1"], np.float32)
        b2 = np.asarray(inputs[f"{br}_b2"], np.float32)
        W0m = (W0 * m0)[:, perm]                     # [32, 256]
        W1m = (W1 * m1)[perm][:, perm]               # [256, 256]
        W2m = (W2 * m2)[perm, :]                     # [256, 32]
        out[f"{br}_w0"] = np.ascontiguousarray(W0m)
        out[f"{br}_w1a"] = np.ascontiguousarray(W1m[:C0, :])        # [128, 256]
        out[f"{br}_w1b"] = np.ascontiguousarray(W1m[C0:, C0:])      # [128, 128]
        sgn = -1.0 if br == "mu" else 1.0
        out[f"{br}_w2a"] = np.ascontiguousarray(sgn * W2m[:C0, :])  # [128, 32]
        out[f"{br}_w2b"] = np.ascontiguousarray(sgn * W2m[C0:, :])  # [128, 32]
        out[f"{br}_b0"] = np.ascontiguousarray(b0[perm]).reshape(H, 1)
        out[f"{br}_b1"] = np.ascontiguousarray(b1[perm]).reshape(H, 1)
        out[f"{br}_b2"] = np.ascontiguousarray(b2).reshape(D, 1)
    # 0.5 * sum(b2_lv): host-side constant added to the logstd reduction
    out["lsb2"] = np.asarray(
        [[0.5 * float(np.sum(np.asarray(inputs["lv_b2"], np.float32)))]],
        np.float32,
    )
    return out


_WEIGHT_SPECS = [
    ("mu_w0", [D, H]), ("mu_w1a", [C0, H]), ("mu_w1b", [C0, C0]),
    ("mu_w2a", [C0, D]), ("mu_w2b", [C0, D]),
    ("mu_b0", [H, 1]), ("mu_b1", [H, 1]), ("mu_b2", [D, 1]),
    ("lv_w0", [D, H]), ("lv_w1a", [C0, H]), ("lv_w1b", [C0, C0]),
    ("lv_w2a", [C0, D]), ("lv_w2b", [C0, D]),
    ("lv_b0", [H, 1]), ("lv_b1", [H, 1]), ("lv_b2", [D, 1]),
    ("lsb2", [1, 1]),
]


# ------------------------------------------------------------- bass builder --
def build_bass(nsteps=D):
    import concourse.bacc as bacc
    import concourse.mybir as mybir
    from concourse.tile import TileContext
    from concourse.masks import make_identity

    f32 = mybir.dt.float32
    f32r = mybir.dt.float32r
    Relu = mybir.ActivationFunctionType.Relu
    Exp = mybir.ActivationFunctionType.Exp
    Copy = mybir.ActivationFunctionType.Copy
    add_op = mybir.AluOpType.add
    max_op = mybir.AluOpType.max
    mult_op = mybir.AluOpType.mult
    sub_op = mybir.AluOpType.subtract

    nc = bacc.Bacc("TRN2", target_bir_lowering=False, debug=False,
                   num_devices=NCORES)

    x_d = nc.declare_dram_parameter("x", [BS, D], f32, isOutput=False)

    def _wdt(nm):
        return f32r if nm.split("_")[-1] in ("w0", "w1a", "w1b", "w2a",
                                             "w2b") else f32
    wd = {nm: nc.declare_dram_parameter(nm, shp, _wdt(nm), isOutput=False)
          for nm, shp in _WEIGHT_SPECS}
    identr_d = nc.declare_dram_parameter("identr", [D, D], f32r, isOutput=False)
    y_d = nc.declare_dram_parameter("y_out", [BS, D], f32, isOutput=True)
    ls_d = nc.declare_dram_parameter("ls_out", [1, BS], f32, isOutput=True)

    with (
        TileContext(nc) as tc,
        tc.tile_pool(name="consts", bufs=1) as cpool,
        tc.tile_pool(name="acts", bufs=1) as apool,
        tc.tile_pool(name="hsb", bufs=4) as hpool,
        tc.tile_pool(name="pmm", bufs=5, space="PSUM") as pmm,
        tc.tile_pool(name="pout", bufs=1, space="PSUM") as pout,
        tc.tile_pool(name="pmisc", bufs=1, space="PSUM") as pmisc,
    ):
        # ---- constants ----
        ident = cpool.tile([C0, C0], f32, tag="ident")
        make_identity(nc, ident[:])
        w = {}
        for nm, shp in _WEIGHT_SPECS:
            if nm.endswith("b0") or nm.endswith("b1"):
                # [256,1] bias -> [128,2]: col 0 = chunk0, col 1 = chunk1
                t = cpool.tile([C0, 2], f32, tag=nm)
                nc.sync.dma_start(
                    out=t[:], in_=wd[nm].rearrange("(c p) o -> p (c o)", c=2))
            else:
                t = cpool.tile(shp, _wdt(nm), tag=nm)
                nc.sync.dma_start(out=t[:], in_=wd[nm][:])
            w[nm] = t
        identr = cpool.tile([D, D], f32r, tag="identr")
        nc.sync.dma_start(out=identr[:], in_=identr_d[:])

        # exp bias: -0.5 * b2_lv  (per-partition [32,1])
        expb = cpool.tile([D, 1], f32, tag="expb")
        nc.scalar.mul(out=expb[:], in_=w["lv_b2"][:], mul=-0.5)

        # 0.5-vector for the final logstd partition reduction
        halves = cpool.tile([D, 1], f32r, tag="halves")
        nc.vector.memset(halves[:].bitcast(f32), 0.5)

        # ---- x: load natural, PE-transpose to xT, fold b2_mu: x1 = xT - b2mu
        x_nat = apool.tile([C0, NB * D], f32, tag="xnat")
        nc.sync.dma_start(
            out=x_nat[:].rearrange("p (c d) -> p c d", c=NB),
            in_=x_d.rearrange("(c p) d -> p c d", p=C0),
        )
        x1 = apool.tile([D, BS], f32r, tag="x1")
        for i in range(NB):
            tp = pmisc.tile([D, C0], f32, tag="misc")
            nc.tensor.transpose(tp[:], x_nat[:, i * D:(i + 1) * D], ident[:])
            nc.vector.tensor_scalar(
                out=x1[:, i * C0:(i + 1) * C0], in0=tp[:],
                scalar1=w["mu_b2"][:], scalar2=None, op0=sub_op,
            )

        # ---- yT state [32, 512], init 0
        yt = apool.tile([D, BS], f32r, tag="yt")
        nc.vector.memset(yt[:].bitcast(f32), 0.0)

        # Exp activation-table preload (off the critical path)
        pre_e = hpool.tile([D, 1], f32, tag="pre_e")
        nc.scalar.activation(pre_e[:], expb[:], Exp)

        lv_ps_last = None
        HB = BS // 2

        # ---------------- the 32 steps ----------------
        # Steps 0..15 only need hidden units with degree <= step (chunk0):
        # output coord k depends on h2 units deg<=k -> h1 units deg<=k ->
        # y coords < k.  Coordinates > k of the y update are garbage-but-
        # carried, exactly like the reference.
        for step in range(nsteps):
            full = step >= 15
            chunks = (0, 1) if full else (0,)

            # mm1: h1T chunks = W0.T @ yT   (K=32, M=128, N=512)
            h1_ps = {}
            for c in chunks:
                for br in ("mu", "lv"):
                    ps = pmm.tile([C0, BS], f32, tag="hps")
                    nc.tensor.matmul(
                        ps[:],
                        w[f"{br}_w0"][:, c * C0:(c + 1) * C0],
                        yt[:],
                        start=True, stop=True,
                    )
                    h1_ps[br, c] = ps

            # relu1 (+b0): mu_c0/lv_c1 on ACT, lv_c0/mu_c1 on DVE
            h1 = {}
            for c in chunks:
                for br in ("mu", "lv"):
                    t = hpool.tile([C0, BS], f32r, tag="h1")
                    on_act = (br == "mu") == (c == 0)
                    if on_act:
                        nc.scalar.activation(
                            t[:], h1_ps[br, c][:], Relu,
                            bias=w[f"{br}_b0"][:, c:c + 1], scale=1.0,
                        )
                    else:
                        nc.vector.tensor_scalar(
                            out=t[:], in0=h1_ps[br, c][:],
                            scalar1=w[f"{br}_b0"][:, c:c + 1], scalar2=0.0,
                            op0=add_op, op1=max_op,
                        )
                    h1[br, c] = t

            # mm2: a2c0 = w1a[:, :128].T @ h1c0
            #      a2c1 = w1a[:, 128:].T @ h1c0 + w1b.T @ h1c1  (full only)
            h2_ps = {}
            for br in ("lv", "mu"):
                ps0 = pmm.tile([C0, BS], f32, tag="hps")
                nc.tensor.matmul(
                    ps0[:], w[f"{br}_w1a"][:, :C0],
                    h1[br, 0][:], start=True, stop=True,
                )
                h2_ps[br, 0] = ps0
            if full:
                for br in ("lv", "mu"):
                    ps1 = pmm.tile([C0, BS], f32, tag="hps")
                    nc.tensor.matmul(
                        ps1[:], w[f"{br}_w1a"][:, C0:],
                        h1[br, 0][:], start=True, stop=False,
                    )
                    nc.tensor.matmul(
                        ps1[:], w[f"{br}_w1b"][:],
                        h1[br, 1][:], start=False, stop=True,
                    )
                    h2_ps[br, 1] = ps1

            # relu2 (+b1)
            h2 = {}
            for c in chunks:
                for br in ("lv", "mu"):
                    t = hpool.tile([C0, BS], f32r, tag="h2")
                    on_act = (br == "mu") == (c == 0)
                    if on_act:
                        nc.scalar.activation(
                            t[:], h2_ps[br, c][:], Relu,
                            bias=w[f"{br}_b1"][:, c:c + 1], scale=1.0,
                        )
                    else:
                        nc.vector.tensor_scalar(
                            out=t[:], in0=h2_ps[br, c][:],
                            scalar1=w[f"{br}_b1"][:, c:c + 1], scalar2=0.0,
                            op0=add_op, op1=max_op,
                        )
                    h2[br, c] = t

            # mm3 lv first (exp is on the critical chain)
            lv_ps = pout.tile([D, BS], f32, tag="olv")
            nc.tensor.matmul(lv_ps[:], w["lv_w2a"][:],
                             h2["lv", 0][:], start=True, stop=not full)
            if full:
                nc.tensor.matmul(lv_ps[:], w["lv_w2b"][:],
                                 h2["lv", 1][:], start=False, stop=True)

            # mm3 mu: psum = x1 - mu  (negated W2mu, then +x1 via identity mm)
            tmu_ps = pout.tile([D, BS], f32, tag="omu")
            nc.tensor.matmul(tmu_ps[:], w["mu_w2a"][:],
                             h2["mu", 0][:], start=True, stop=False)
            if full:
                nc.tensor.matmul(tmu_ps[:], w["mu_w2b"][:],
                                 h2["mu", 1][:], start=False, stop=False)
            nc.tensor.matmul(tmu_ps[:], identr[:],
                             x1[:], start=False, stop=True)

            # tail in batch halves: e = exp(-0.5*lv - 0.5*b2lv); y = t * e
            e = hpool.tile([D, BS], f32, tag="e")
            for hh in range(2):
                sl = slice(hh * HB, (hh + 1) * HB)
                nc.scalar.activation(e[:, sl], lv_ps[:, sl], Exp,
                                     bias=expb[:], scale=-0.5)
                nc.vector.tensor_tensor(out=yt[:, sl], in0=tmu_ps[:, sl],
                                        in1=e[:, sl], op=mult_op)

            lv_ps_last = lv_ps

        # ---------------- outputs ----------------
        # logstd_sum = 0.5 * sum_d lv_last[d, b] + 0.5*sum(b2lv)
        lvt = hpool.tile([D, BS], f32r, tag="lvt")
        nc.scalar.activation(lvt[:], lv_ps_last[:], Copy)
        ls_ps = pmisc.tile([1, BS], f32, tag="misc")
        nc.tensor.matmul(ls_ps[:], halves[:], lvt[:], start=True, stop=True)
        ls_sb = hpool.tile([1, BS], f32, tag="lssb")
        nc.vector.tensor_scalar(
            out=ls_sb[:], in0=ls_ps[:], scalar1=w["lsb2"][0:1, 0:1],
            scalar2=None, op0=add_op,
        )
        nc.sync.dma_start(out=ls_d[:], in_=ls_sb[:])

        # y: PE-transpose yT [32,512] -> [512,32] and DMA out
        y_nat = hpool.tile([C0, NB * D], f32, tag="ynat")
        for i in range(NB):
            yps = pmisc.tile([C0, D], f32, tag="misc")
            nc.tensor.transpose(yps[:], yt[:, i * C0:(i + 1) * C0].bitcast(f32),
                                ident[:D, :D])
            nc.scalar.activation(y_nat[:, i * D:(i + 1) * D], yps[:], Copy)
        nc.sync.dma_start(
            out=y_d.rearrange("(c p) d -> p c d", p=C0),
            in_=y_nat[:].rearrange("p (c d) -> p c d", c=NB),
        )

    nc.finalize()
    return nc


# ------------------------------------------------------------ host entry ----
_CACHED = {}


def make_in_maps(inputs):
    x = np.asarray(inputs["x"], np.float32)
    wmaps = _prep_weights(inputs)
    ident = np.eye(D, dtype=np.float32)
    in_maps = []
    for c in range(NCORES):
        m = {"x": np.ascontiguousarray(x[c * BS:(c + 1) * BS]),
             "identr": ident}
        m.update(wmaps)
        in_maps.append(m)
    return in_maps


def kernel(**inputs):
    import concourse.bass_utils as bass_utils

    if "nc" not in _CACHED:
        _CACHED["nc"] = build_bass()
    nc = _CACHED["nc"]
    in_maps = make_in_maps(inputs)

    res = bass_utils.run_bass_kernel_spmd(nc, in_maps,
                                          core_ids=list(range(NCORES)))
    y = np.concatenate([res.results[c]["y_out"] for c in range(NCORES)], axis=0)
    ls = np.concatenate(
        [res.results[c]["ls_out"].reshape(BS) for c in range(NCORES)], axis=0
    )
    return y, ls


if __name__ == "__main__":
    nc = build_bass()
    print("built ok")
